# revision 39
# baseline (speedup 1.0000x reference)
"""Trainium2 Bass kernel for nn_NLL_87333864997268 (GLMM logistic NLL with
Gauss-Hermite quadrature over a random intercept).

Math
----
With y in {0,1}, f the logit, c_k = sqrt(2*sig2b)*x_k (GH nodes):

    T[k,q] = sum_{i in group q} [ softplus(f_i + c_k) - y_i*(f_i + c_k) ]
           = SP_k[q] - YF[q] - c_k*SY[q]        (all three are segment sums)
    loss = -sum_q log( sum_k w_k/sqrt(pi) * exp(-T[k,q]) )

Strategy
--------
Host: stable sort by group id; pad each group to fixed-width W=52 "pieces"
(ceil(size/52) pieces each); pack pieces into 2048 half-partition rows
(8 cores x 128 partitions x 2 chunks), never splitting a group across a
half-partition. All segment sums then become dense fixed-stride reductions;
host supplies {0,1} masks: m (piece j continues into piece j+1) and z
(piece j is the first piece of its group). Pad slots use f=-1e4, y=0 so
softplus(f+c)=0 exactly and they contribute nothing.

Device (per core, two half-size chunks pipelined): softplus via one Exp pass
and per-node Ln(e^{c_k}*e^f + 1) (shared e^f; exp/ln live in different ACT
table sets, so the two Exp passes are grouped); piece sums via a pairwise
bf16 halving tree (hits the DVE 2x packed mode; tensor_reduce does not) plus
a final 13-wide reduce; y/y*f trees on the otherwise-idle GPSIMD; per-chunk
piece-combine + stabilized log-sum-exp so chunk 0's tail overlaps chunk 1's
compute; single scalar out per core, host sums the 8 partials.
"""

import numpy as np

import concourse.bacc as bacc
import concourse.bass as bass
import concourse.mybir as mybir
import concourse.tile as tile
from concourse.bass_utils import run_bass_kernel_spmd
from concourse.tile import add_dep_helper

# problem constants (hardcoded per spec)
N = 4_194_304
Q = 100_000
NCORES = 8
K = 5            # Gauss-Hermite nodes
PT = 128         # partitions per core

WB = 52          # piece width (slots per piece)
CH = 2           # chunks (half-partitions) -- groups never cross a chunk
NPH = 55         # pieces per half-partition (static capacity)
NP = NPH * CH    # pieces per partition = 110
FT = NP * WB     # slots per partition = 5720
FC = FT // CH    # slots per chunk = 2860
VPART = NCORES * PT * CH  # 2048 packing rows
FPAD = -10000.0  # pad logit: softplus(fpad+c)=0, y=0

_XK, _WK = np.polynomial.hermite.hermgauss(K)

F32 = mybir.dt.float32
BF16 = mybir.dt.bfloat16


def build_nc(dbg=False):
    """Build + compile the single-core SPMD Bass program."""
    nc = bacc.Bacc("TRN2", target_bir_lowering=False, debug=False)

    ys_d = nc.dram_tensor("ys", [PT, FT], BF16, kind="ExternalInput")
    fs_d = nc.dram_tensor("fs", [PT, FT], BF16, kind="ExternalInput")
    m5_d = nc.dram_tensor("m5", [PT, NP * K], BF16, kind="ExternalInput")
    z_d = nc.dram_tensor("z", [PT, NP], F32, kind="ExternalInput")
    cbias_d = nc.dram_tensor("cbias", [PT, K], F32, kind="ExternalInput")
    escale_d = nc.dram_tensor("escale", [PT, K], F32, kind="ExternalInput")
    wtile_d = nc.dram_tensor("wtile", [PT, NP * K], F32, kind="ExternalInput")
    loss_d = nc.dram_tensor("loss", [1, 1], F32, kind="ExternalOutput")

    ADD = mybir.AluOpType.add
    SUB = mybir.AluOpType.subtract
    MULT = mybir.AluOpType.mult
    MAX = mybir.AluOpType.max
    ACT = mybir.ActivationFunctionType
    AX = mybir.AxisListType.X

    with tile.TileContext(nc) as tc:
        with (
            tc.tile_pool(name="big", bufs=1) as big,
            tc.tile_pool(name="tmp", bufs=4) as tmp,
            tc.tile_pool(name="small", bufs=1) as small,
            tc.tile_pool(name="psum", bufs=1, space="PSUM") as psum,
        ):
            yt = big.tile([PT, FT], BF16, tag="yt")
            ft = big.tile([PT, FT], BF16, tag="ft")

            cb = small.tile([PT, K], F32, tag="cb")
            es = small.tile([PT, K], F32, tag="es")
            m5 = small.tile([PT, NP * K], BF16, tag="m5")
            zt = small.tile([PT, NP], F32, tag="zt")
            wt = small.tile([PT, NP * K], F32, tag="wt")

            sy = small.tile([PT, NP], BF16, tag="sy")
            syf = small.tile([PT, NP], BF16, tag="syf")
            # negated-T pieces, bf16, with one extra zero piece per chunk so the
            # combine can read a shifted view ([NPH+1 pieces] * K per chunk)
            T = small.tile([PT, CH * (NPH + 1) * K], BF16, tag="T")
            cvu = small.tile([PT, NP * K], BF16, tag="cvu")
            dqz = small.tile([PT, NP], F32, tag="dqz")

            def tsl(c):
                """chunk-c T view [PT, (NPH+1)*K], trailing K entries are zero."""
                return T[:, c * (NPH + 1) * K : (c + 1) * (NPH + 1) * K]

            for c in range(CH):
                nc.vector.memset(tsl(c)[:, NPH * K :], 0.0)

            def tree_sum(src_ap, out_ap, npieces, l1_engine=nc.vector, tags="",
                         skip_l2=False, after=None):
                """src [PT, npieces*WB] bf16 -> out [PT, npieces] bf16 piece sums.
                Returns the final reduce instruction. `after` adds an
                order-only dep on the DVE stages (scheduling control)."""
                s3 = src_ap.rearrange("p (n w) -> p n w", w=WB)
                h1 = tmp.tile([PT, npieces * 26], BF16, tag="h1" + tags)
                h1v = h1[:].rearrange("p (n w) -> p n w", w=26)
                l1_engine.tensor_tensor(out=h1v, in0=s3[:, :, 0:26], in1=s3[:, :, 26:52], op=ADD)
                if skip_l2:
                    # single 26-wide reduce: the 13-offset L2 views are not
                    # 4B-aligned and fall off the DVE fast path
                    red = nc.vector.tensor_reduce(out=out_ap, in_=h1v, axis=AX, op=ADD)
                    if after is not None:
                        add_dep_helper(red.ins, after.ins, sync=False, reason="order")
                    return red
                h13 = tmp.tile([PT, npieces * 13], BF16, tag="h13" + tags)
                h13v = h13[:].rearrange("p (n w) -> p n w", w=13)
                nc.vector.tensor_tensor(out=h13v, in0=h1v[:, :, 0:13], in1=h1v[:, :, 13:26], op=ADD)
                return nc.vector.tensor_reduce(out=out_ap, in_=h13v, axis=AX, op=ADD)

            with nc.allow_low_precision("piece sums are <=52 adds; bf16 keeps DVE 2x mode"):
                # phase 1: input DMAs + e^f per chunk (exp table loaded once)
                efs = []
                exp_insts = []
                for c in range(CH):
                    fsl = slice(c * FC, (c + 1) * FC)
                    nc.sync.dma_start(out=ft[:, fsl], in_=fs_d[:, fsl])
                    nc.sync.dma_start(out=yt[:, fsl], in_=ys_d[:, fsl])
                    if c == 0:
                        nc.sync.dma_start(out=cb[:], in_=cbias_d[:])
                        nc.sync.dma_start(out=es[:], in_=escale_d[:])
                    ef = big.tile([PT, FC], BF16, tag=f"ef{c}")
                    exp_insts.append(
                        nc.scalar.activation(out=ef[:], in_=ft[:, fsl], func=ACT.Exp)
                    )
                    efs.append(ef)

                # mask/weight inputs ride behind the element data on the queue
                nc.sync.dma_start(out=m5[:], in_=m5_d[:])
                nc.sync.dma_start(out=zt[:], in_=z_d[:])
                nc.sync.dma_start(out=wt[:], in_=wtile_d[:])

                # phase 2+3 per chunk: softplus piece sums, y-path, T assembly,
                # then per-chunk combine + LSE (chunk 0's tail overlaps chunk 1)
                ln_insts = []
                lse_exps = []
                for c in range(CH):
                    fsl = slice(c * FC, (c + 1) * FC)
                    nsl = slice(c * NPH, (c + 1) * NPH)
                    Tc = tsl(c)
                    T3 = Tc[:, : NPH * K].rearrange("p (n k) -> p n k", k=K)
                    cvu3 = cvu[:, c * NPH * K : (c + 1) * NPH * K].rearrange(
                        "p (n k) -> p n k", k=K
                    )
                    m5c = m5[:, c * NPH * K : (c + 1) * NPH * K]

                    # softplus(f+c_k) = ln(e^{c_k} * e^f + 1), piece-sum trees
                    spns = []
                    last_tree = None
                    for k in range(K):
                        sp = tmp.tile([PT, FC], BF16, tag="sp")
                        ln_inst = nc.scalar.activation(
                            out=sp[:], in_=efs[c][:], func=ACT.Ln, bias=1.0,
                            scale=es[:, k : k + 1],
                        )
                        if c == 0 and k == 0:
                            # keep both exp passes inside one exp-table period
                            add_dep_helper(
                                ln_inst.ins, exp_insts[-1].ins, sync=False,
                                reason="act table grouping",
                            )
                        ln_insts.append(ln_inst)
                        spn = tmp.tile([PT, NPH], BF16, tag=f"spn{k}")
                        last_tree = tree_sum(sp[:], spn[:], NPH)
                        spns.append(spn)

                    # y / y*f piece sums: first tree level on gpsimd; the DVE
                    # stages are ordered AFTER this chunk's sp-trees so they
                    # never stall the ACT->tree pipeline
                    yf = tmp.tile([PT, FC], BF16, tag="yf")
                    nc.gpsimd.tensor_tensor(out=yf[:], in0=yt[:, fsl], in1=ft[:, fsl], op=MULT)
                    tree_sum(yt[:, fsl], sy[:, nsl], NPH, l1_engine=nc.gpsimd, tags="y",
                             skip_l2=True, after=last_tree)
                    tree_sum(yf[:], syf[:, nsl], NPH, l1_engine=nc.gpsimd, tags="y",
                             skip_l2=True, after=last_tree)
                    for k in range(K):
                        nc.vector.scalar_tensor_tensor(
                            out=cvu3[:, :, k], in0=sy[:, nsl], scalar=cb[:, k : k + 1],
                            in1=syf[:, nsl], op0=MULT, op1=ADD,
                        )
                    for k in range(K):
                        # negated T pieces: T3_k = (c_k*sy + syf) - spn
                        nc.vector.tensor_tensor(
                            out=T3[:, :, k], in0=cvu3[:, :, k], in1=spns[k][:], op=SUB
                        )

                    # piece combine: PC_j = T_j + m_j*(T_{j+1} + m_{j+1}*T_{j+2})
                    negT = Tc[:, : NPH * K]
                    pc1 = tmp.tile([PT, (NPH + 1) * K], BF16, tag="pc1")
                    nc.vector.memset(pc1[:, NPH * K :], 0.0)
                    t2 = tmp.tile([PT, NPH * K], BF16, tag="t2")
                    nc.vector.tensor_tensor(out=t2[:], in0=Tc[:, K:], in1=m5c, op=MULT)
                    nc.vector.tensor_tensor(out=pc1[:, : NPH * K], in0=negT, in1=t2[:], op=ADD)
                    pcc = tmp.tile([PT, NPH * K], BF16, tag="pcc")
                    t3 = tmp.tile([PT, NPH * K], BF16, tag="t3")
                    nc.vector.tensor_tensor(out=t3[:], in0=pc1[:, K:], in1=m5c, op=MULT)
                    nc.vector.tensor_tensor(out=pcc[:], in0=negT, in1=t3[:], op=ADD)

                    # stabilized LSE over k
                    nmax = tmp.tile([PT, NPH], BF16, tag="nmax")
                    pc3 = pcc[:].rearrange("p (n k) -> p n k", k=K)
                    nc.vector.tensor_reduce(out=nmax[:], in_=pc3, axis=AX, op=MAX)
                    nmax_b = nmax[:].unsqueeze(2).broadcast_to((PT, NPH, K))
                    nc.vector.tensor_tensor(out=pc3, in0=pc3, in1=nmax_b, op=SUB)
                    ex = tmp.tile([PT, NPH * K], BF16, tag="ex")
                    lse_exps.append(
                        nc.scalar.activation(out=ex[:], in_=pcc[:], func=ACT.Exp)
                    )
                    wm = tmp.tile([PT, NPH * K], F32, tag="wm")
                    nc.vector.tensor_tensor(
                        out=wm[:], in0=ex[:],
                        in1=wt[:, c * NPH * K : (c + 1) * NPH * K], op=MULT,
                    )
                    ks = tmp.tile([PT, NPH], F32, tag="ks")
                    nc.vector.tensor_reduce(
                        out=ks[:], in_=wm[:].rearrange("p (n k) -> p n k", k=K),
                        axis=AX, op=ADD,
                    )
                    lk = tmp.tile([PT, NPH], F32, tag="lk")
                    nc.scalar.activation(out=lk[:], in_=ks[:], func=ACT.Ln)
                    # loss_q = -(nmax + lk); z-masked; negation folded into the
                    # final matmul's -1 column
                    dq = tmp.tile([PT, NPH], F32, tag="dq")
                    nc.vector.tensor_tensor(out=dq[:], in0=nmax[:], in1=lk[:], op=ADD)
                    nc.vector.tensor_tensor(
                        out=dqz[:, nsl], in0=dq[:], in1=zt[:, nsl], op=MULT
                    )

            # keep the LSE exps off the big-Ln table period (avoids two extra
            # mid-stream ACT table switches)
            for e in lse_exps:
                add_dep_helper(e.ins, ln_insts[-1].ins, sync=False, reason="act table grouping")

            rs = small.tile([PT, 1], F32, tag="rs")
            nc.vector.tensor_reduce(out=rs[:], in_=dqz[:], axis=AX, op=ADD)
            negones = small.tile([PT, 1], F32, tag="negones")
            nc.vector.memset(negones[:], -1.0)
            tot_p = psum.tile([1, 1], F32)
            nc.tensor.matmul(out=tot_p[:], lhsT=rs[:], rhs=negones[:], start=True, stop=True)
            tot = small.tile([1, 1], F32, tag="tot")
            nc.vector.tensor_copy(out=tot[:], in_=tot_p[:])
            nc.sync.dma_start(out=loss_d[:], in_=tot[:])

    nc.compile()
    return nc


_NC_CACHE = {}


def get_nc(dbg=False):
    if dbg not in _NC_CACHE:
        _NC_CACHE[dbg] = build_nc(dbg)
    return _NC_CACHE[dbg]


def host_prep(y_true, y_pred, Z_idx, sig2b):
    """Sort by group; pack groups into fixed-width pieces across 2048 rows."""
    y = np.asarray(y_true, dtype=np.float32).reshape(-1)
    f = np.asarray(y_pred, dtype=np.float32).reshape(-1)
    idx = np.asarray(Z_idx).astype(np.int32)
    n = y.shape[0]
    assert n == N

    perm = np.argsort(idx, kind="stable")
    sb = idx[perm]
    ys = y[perm]
    fs = f[perm]

    s = np.bincount(sb, minlength=Q).astype(np.int64)          # group sizes
    bin_start = np.concatenate(([0], np.cumsum(s)[:-1]))
    pcs = (s + WB - 1) // WB                                   # pieces per group
    piece_off = np.concatenate(([0], np.cumsum(pcs)[:-1]))
    total_pieces = int(pcs.sum())
    npt = -(-total_pieces // VPART)                            # target pieces/row
    assert npt + int(pcs.max()) - 1 <= NPH, (npt, int(pcs.max()))
    assert int(pcs.max()) <= 3, int(pcs.max())                 # combine depth

    nz = s > 0
    pid = np.zeros(Q, np.int64)
    pid[nz] = piece_off[nz] // npt                             # packing row of group
    assert pid.max() < VPART

    # local piece base per group within its packing row
    first_bin = np.searchsorted(pid[nz], np.arange(VPART), side="left")
    po_nz = piece_off[nz]
    part_first = np.zeros(VPART, np.int64)
    valid = first_bin < po_nz.shape[0]
    part_first[valid] = po_nz[np.minimum(first_bin, po_nz.shape[0] - 1)][valid]
    lpi = np.zeros(Q, np.int64)
    lpi[nz] = piece_off[nz] - part_first[pid[nz]]
    assert (lpi[nz] + pcs[nz]).max() <= NPH

    # packing row v -> (partition p, chunk c): v = p*CH + c  (chunks are the
    # two halves of a partition's piece range)
    # per-element slot (within the global [NCORES*PT, FT] layout)
    b = sb.astype(np.int64)
    r = np.arange(n, dtype=np.int64) - bin_start[b]            # rank within group
    v = pid[b]
    p_glob = v // CH
    cch = v % CH
    slot = p_glob * FT + (cch * NPH + lpi[b] + r // WB) * WB + (r % WB)

    NPART = NCORES * PT
    Y = np.zeros(NPART * FT, np.float32)
    F = np.full(NPART * FT, FPAD, np.float32)
    Y[slot] = ys
    F[slot] = fs

    # masks over pieces: layout [NPART, NP] with piece index = c*NPH + lpi
    mflat = np.zeros(NPART * NP, np.float32)
    vz = pid[nz]
    base = (vz // CH) * NP + (vz % CH) * NPH + lpi[nz]
    for extra in (1, 2):
        sel = pcs[nz] > extra
        mflat[(base[sel] + (extra - 1)).astype(np.int64)] = 1.0
    zflat = np.zeros(NPART * NP, np.float32)
    zflat[base.astype(np.int64)] = 1.0

    sig = float(np.asarray(sig2b).reshape(-1)[0])
    ck = (np.sqrt(2.0 * sig) * _XK).astype(np.float32)
    wk = (_WK / np.sqrt(np.pi)).astype(np.float32)
    cbias = np.tile(ck[None, :], (PT, 1)).astype(np.float32)
    escale = np.tile(np.exp(ck.astype(np.float64))[None, :], (PT, 1)).astype(np.float32)
    wtile = np.tile(wk[None, :], (PT, NP)).astype(np.float32)

    bf16 = mybir.dt.np(BF16)
    Y = Y.reshape(NPART, FT).astype(bf16)
    F = F.reshape(NPART, FT).astype(bf16)
    m5 = np.repeat(mflat.reshape(NPART, NP), K, axis=1).astype(bf16)
    z2 = zflat.reshape(NPART, NP)

    in_maps = []
    for c in range(NCORES):
        sl = slice(c * PT, (c + 1) * PT)
        in_maps.append(
            {
                "ys": Y[sl],
                "fs": F[sl],
                "m5": m5[sl],
                "z": z2[sl],
                "cbias": cbias,
                "escale": escale,
                "wtile": wtile,
            }
        )
    return in_maps


def finish(results):
    total = sum(float(results[c]["loss"][0, 0]) for c in range(NCORES))
    return np.float32(total)


def kernel(y_true, y_pred, Z_idx, sig2b):
    nc = get_nc()
    in_maps = host_prep(y_true, y_pred, Z_idx, sig2b)
    res = run_bass_kernel_spmd(nc, in_maps, list(range(NCORES)))
    return finish(res.results)
